# revision 16
# baseline (speedup 1.0000x reference)
"""Trainium2 Bass kernel for nn_GroupedConvFuseSide4.

out[b,k] = w[k,0]*side5[b,k] + w[k,1]*side4[b,k]
         + w[k,2]*side1[b,0] + w[k,3]*side2[b,0] + w[k,4]*side3[b,0] + bias[k]

Sharding: pure data parallel over batch (B=8) across 8 NeuronCores.

Per-core scheme (fp16 staging, 128-partition packed pairs): the op is
memory-bound, so all large tensors are staged in DRAM as fp16 (host converts;
rel-err ~1e-3 vs the 2e-2 gate). The 262144 pixels of one image are split
into CH=32 chunks of FD=8192. The (chunk, k) pairs are enumerated
chunk-major into 608 rows; tiles take 128 consecutive rows (4 full tiles +
a 96-row tail), so every side5/side4/out DMA is one contiguous
[128, 16KB] = 2MB transfer with full 16-engine fanout.

Per tile: PE matmul (contraction = ones row + 3 singles x nct chunks, fp16,
zero-padded to a fixed 25 rows) computes base = w2*s1 + w3*s2 + w4*s3 + bias
into fp32 PSUM; the scalar engine (ACT) evacuates PSUM to fp16 SBUF; DVE
merges side5/side4 with tensor_scalar_mul (4x packed mode) + tensor_add
(2x) — scalar_tensor_tensor is avoided since it only runs in 1x mode.
The ones rows ride along in the singles DMA (no DVE memset). Weights/bias/
lhsT are baked into the program as inline const tensors.
"""

import numpy as np

B, K, H, W = 8, 19, 512, 512
NPIX = H * W               # 262144 pixels per channel image
FD = 4096                  # pixels per chunk
CH = NPIX // FD            # 32 chunks per image
NPAIR = CH * K             # 608 (chunk, k) pairs, chunk-major: i -> (i//K, i%K)
PT = 128                   # partitions per full tile
NT = NPAIR // PT           # 4 full tiles
TAIL = NPAIR - NT * PT     # 96-row tail tile
NTT = NT + 1               # 5 tiles total
MAXR = 25                  # fixed contraction rows (1 ones + 3*8 singles max)
CC = 2048                  # compute chunk (4 PSUM banks fp32)
N_CORES = 8

# per-tile geometry: (row offset, partitions, first chunk, n chunks)
_TILES = []
for _t in range(NTT):
    _i0 = _t * PT
    _pt = PT if _t < NT else TAIL
    _c0 = _i0 // K
    _c1 = (_i0 + _pt - 1) // K
    _TILES.append((_i0, _pt, _c0, _c1 - _c0 + 1))

_XS_OFF = []               # row offsets of each tile's block in xsall
_o = 0
for _, _, _, _nct in _TILES:
    _XS_OFF.append(_o)
    _o += 1 + 3 * _nct     # ones row + singles rows
XS_ROWS = _o               # 113

_cache = {}


def _build_program(w, b):
    import concourse.bacc as bacc
    import concourse.tile as tile
    import concourse.mybir as mybir
    from contextlib import ExitStack

    f16 = mybir.dt.float16
    f32 = mybir.dt.float32
    mult = mybir.AluOpType.mult
    add = mybir.AluOpType.add

    nc = bacc.Bacc(
        "TRN2", target_bir_lowering=False, debug=False,
        enable_asserts=False, num_devices=N_CORES,
    )

    x5d = nc.dram_tensor("x5", [NPAIR, FD], f16, kind="ExternalInput").ap()
    x4d = nc.dram_tensor("x4", [NPAIR, FD], f16, kind="ExternalInput").ap()
    xsd = nc.dram_tensor("xs", [XS_ROWS, FD], f16, kind="ExternalInput").ap()
    outd = nc.dram_tensor("out", [NPAIR, FD], f16, kind="ExternalOutput").ap()

    # ---- baked constants, consolidated into three inline tensors ----
    lhsT_all = np.zeros((MAXR, NTT * PT), dtype=np.float16)
    w0_all = np.zeros((PT, NTT), dtype=np.float32)
    w1_all = np.zeros((PT, NTT), dtype=np.float32)
    for t, (i0, pt, c0, nct) in enumerate(_TILES):
        for p in range(pt):
            i = i0 + p
            ch, k = i // K, i % K
            j = ch - c0
            col = t * PT + p
            lhsT_all[0, col] = b[k]
            lhsT_all[1 + 0 * nct + j, col] = w[k, 2]
            lhsT_all[1 + 1 * nct + j, col] = w[k, 3]
            lhsT_all[1 + 2 * nct + j, col] = w[k, 4]
            w0_all[p, t] = w[k, 0]
            w1_all[p, t] = w[k, 1]
    lhsT_d = nc.inline_tensor(lhsT_all, name="lhsT").ap()
    w0_d = nc.inline_tensor(w0_all, name="w0v").ap()
    w1_d = nc.inline_tensor(w1_all, name="w1v").ap()

    with tile.TileContext(nc) as tc, ExitStack() as ctx:
        consts = ctx.enter_context(tc.tile_pool(name="consts", bufs=1))
        xs_pool = ctx.enter_context(tc.tile_pool(name="xs", bufs=1))
        x5_pool = ctx.enter_context(tc.tile_pool(name="x5", bufs=4))
        x4_pool = ctx.enter_context(tc.tile_pool(name="x4", bufs=4))
        b_pool = ctx.enter_context(tc.tile_pool(name="bb", bufs=3))
        o_pool = ctx.enter_context(tc.tile_pool(name="o", bufs=3))
        psum_pool = ctx.enter_context(tc.tile_pool(name="ps", bufs=2, space="PSUM"))

        lt = consts.tile([MAXR, NTT * PT], f16, tag="lt")
        w0t = consts.tile([PT, NTT], f32, tag="w0t")
        w1t = consts.tile([PT, NTT], f32, tag="w1t")
        nc.sync.dma_start(out=lt[:], in_=lhsT_d)
        nc.sync.dma_start(out=w0t[:], in_=w0_d)
        nc.sync.dma_start(out=w1t[:], in_=w1_d)

        # singles tiles (ring of 2); row 0 = ones (comes in via the DMA)
        xs_tiles = [xs_pool.tile([MAXR, FD], f16, tag=f"xs{i}", name=f"xs{i}")
                    for i in range(3)]

        # software-pipelined: tile t's final add + store are emitted during
        # tile t+1, so DVE never stalls waiting for the PSUM evacuation.
        pend = None            # (u_tile, b16_tile, i0, pt) awaiting final add

        def flush_pend():
            nonlocal pend
            if pend is None:
                return
            u, b16, pi0, ppt = pend
            o = o_pool.tile([PT, FD], f16, tag="o", name="o")
            nc.vector.tensor_add(o[:ppt, :], u[:ppt, :], b16[:ppt, :])
            nc.gpsimd.dma_start(out=outd[pi0:pi0 + ppt], in_=o[:ppt, :])
            pend = None

        for t in range(NTT):
            i0, pt, c0, nct = _TILES[t]
            rows = 1 + 3 * nct
            xs = xs_tiles[t % 3]

            xo = _XS_OFF[t]
            nc.sync.dma_start(out=xs[0:rows, :], in_=xsd[xo:xo + rows])
            x5 = x5_pool.tile([PT, FD], f16, tag="x5")
            nc.sync.dma_start(out=x5[:pt, :], in_=x5d[i0:i0 + pt])
            x4 = x4_pool.tile([PT, FD], f16, tag="x4")
            nc.sync.dma_start(out=x4[:pt, :], in_=x4d[i0:i0 + pt])

            # full-width in-place DVE ops (amortize the per-instruction bubble)
            nc.vector.tensor_scalar_mul(
                x5[:pt, :], x5[:pt, :], w0t[:pt, t:t + 1])
            nc.vector.tensor_scalar_mul(
                x4[:pt, :], x4[:pt, :], w1t[:pt, t:t + 1])
            nc.vector.tensor_add(x4[:pt, :], x5[:pt, :], x4[:pt, :])

            b16 = b_pool.tile([PT, FD], f16, tag="b16")
            for c in range(FD // CC):
                sl = slice(CC * c, CC * (c + 1))
                ps = psum_pool.tile([PT, CC], f32, tag="ps")
                for m in range(CC // 512):
                    msl = slice(CC * c + 512 * m, CC * c + 512 * (m + 1))
                    nc.tensor.matmul(
                        ps[:pt, 512 * m:512 * (m + 1)],
                        lt[:rows, t * PT:t * PT + pt],
                        xs[:rows, msl],
                        start=True, stop=True,
                    )
                nc.scalar.copy(b16[:pt, sl], ps[:pt, :])

            flush_pend()
            pend = (x4, b16, i0, pt)

        flush_pend()

    nc.compile()
    return nc


def _get_program(w, b):
    key = (w.tobytes(), b.tobytes())
    if key not in _cache:
        _cache[key] = _build_program(w, b)
    return _cache[key]


def _pack_pairs(a):
    """[K, CH, FD] fp16 -> [NPAIR, FD] in chunk-major (chunk, k) pair order."""
    return np.ascontiguousarray(a.transpose(1, 0, 2).reshape(NPAIR, FD))


def run(inputs, trace=False, tmpdir=None):
    from concourse.bass_utils import run_bass_kernel_spmd

    w = np.asarray(inputs["weight"], dtype=np.float32)
    b = np.asarray(inputs["bias"], dtype=np.float32)
    nc = _get_program(w, b)

    s1f = np.asarray(inputs["side1"], dtype=np.float16).reshape(B, CH, FD)
    s2f = np.asarray(inputs["side2"], dtype=np.float16).reshape(B, CH, FD)
    s3f = np.asarray(inputs["side3"], dtype=np.float16).reshape(B, CH, FD)
    s4f = np.asarray(inputs["side4"], dtype=np.float16).reshape(B, K, CH, FD)
    s5f = np.asarray(inputs["side5"], dtype=np.float16).reshape(B, K, CH, FD)
    ones = np.ones((1, FD), dtype=np.float16)

    in_maps = []
    for c in range(N_CORES):
        xs_blocks = []
        for _, _, c0, nct in _TILES:
            xs_blocks += [ones, s1f[c, c0:c0 + nct], s2f[c, c0:c0 + nct],
                          s3f[c, c0:c0 + nct]]
        in_maps.append({
            "x5": _pack_pairs(s5f[c]),
            "x4": _pack_pairs(s4f[c]),
            "xs": np.ascontiguousarray(np.concatenate(xs_blocks, axis=0)),
        })

    res = run_bass_kernel_spmd(nc, in_maps, list(range(N_CORES)),
                               trace=trace, tmpdir=tmpdir)
    outs = []
    for c in range(N_CORES):
        o = res.results[c]["out"]                      # [NPAIR, FD] fp16
        o = o.reshape(CH, K, FD).transpose(1, 0, 2)    # [K, CH, FD]
        outs.append(o.reshape(1, K, H, W).astype(np.float32))
    return np.concatenate(outs, axis=0), res


def kernel(**inputs):
    out, _ = run(inputs, trace=False)
    return out
